# revision 19
# baseline (speedup 1.0000x reference)
"""Trainium2 Bass kernel for nn_Encoder_21715354649978 (THP-style encoder).

Contract: kernel(**inputs) takes the FULL unsharded inputs (as produced by
reference.setup_inputs()) and returns the full (enc, sim) outputs.

Sharding: 8 cores = (batch b in 0..3) x (query-tile parity in 0..1).
Each core processes 8 query tiles of 128 tokens (the even or odd global
tiles of its batch) and the full key sequence.  A/W vertex tables are
row-sharded per core by the data-dependent row set its queries gather
(the pairwise gather reads random rows).  One SPMD program runs on all 8
cores; every per-core difference is input data (including the causal
masks, which are data-driven compares against host-provided query ids).
"""

import numpy as np
import ml_dtypes

B, S = 4, 2048
DM, H, DK, DV, DI = 512, 4, 128, 128, 1024
NT, NV = 64, 5000

NVP = 5120          # padded A/W row length (cols 5000.. are zeros)
ZCOL = 5100         # column redirect for pad keys (always-zero column)
RMAX = 1032         # compacted table rows (<=1025 distinct incl. zero row)
NTPC = 8            # own query tiles per core
NGT = 16            # global tiles
NEG = -1.0e9
BF16 = ml_dtypes.bfloat16

_cache = {}


def _wrap16(idx):
    """[N] ints -> [128, N//16] int16 in the wrapped-16 + replicated layout
    used by dma_gather / ap_gather (unwrapped n = s*16 + p per 16-row group).
    """
    idx = np.asarray(idx)
    a = idx.reshape(-1, 16).T.astype(np.int16)   # [16, N//16]
    return np.ascontiguousarray(np.tile(a, (8, 1)))


# ---------------------------------------------------------------------------
# SPMD program (identical for all 8 cores; all per-core variation is data)
# ---------------------------------------------------------------------------

def _build_program(dbg=False):
    import concourse.bass as bass  # noqa: F401
    import concourse.mybir as mybir
    import concourse.tile as tile
    from concourse import bacc
    from concourse.masks import make_identity

    dt = mybir.dt
    Alu = mybir.AluOpType
    Act = mybir.ActivationFunctionType
    AX = mybir.AxisListType
    f32, bf16, i16 = dt.float32, dt.bfloat16, dt.int16

    nc = bacc.Bacc("TRN2", target_bir_lowering=False, debug=False,
                   enable_asserts=False)

    def din(name, shape, dtype):
        return nc.dram_tensor(name, list(shape), dtype, kind="ExternalInput").ap()

    def dout(name, shape, dtype):
        return nc.dram_tensor(name, list(shape), dtype, kind="ExternalOutput").ap()

    Ac = din("Ac", (RMAX, NVP), f32)            # rows of 10*A, compacted
    Wc = din("Wc", (RMAX, NVP), f32)            # rows of W, same compaction
    i32 = dt.int32
    row_idx = din("row_idx", (128, NTPC), i32)  # table row per (p, s)
    col_idx = din("col_idx", (128, 128), i16)   # wrapped-16 for ap_gather
    ev_idx = din("ev_idx", (128, NGT), i32)     # global token order
    vx_idx = din("vx_idx", (128, NGT), i32)
    ev_idx_o = din("ev_idx_o", (128, NTPC), i32)  # own tokens, s-order
    vx_idx_o = din("vx_idx_o", (128, NTPC), i32)
    ev_emb = din("ev_emb", (NT + 1, DM), f32)
    vx_emb = din("vx_emb", (NV + 1, DM), f32)
    tvals_g = din("tvals_g", (128, NGT), f32)
    tvals_o = din("tvals_o", (128, NTPC), f32)
    ivals = din("ivals", (128, NTPC), f32)      # global query index per (p,s)
    jkrow = din("jkrow", (128, S), f32)         # j, or 3e9 for pad keys
    vmvals = din("vmvals", (128, NTPC), f32)
    ipv2 = din("ipv2", (128, DM), f32)      # invpv / (2*pi)
    off2 = din("off2", (128, DM), f32)      # phase offset / (2*pi)
    off2m = din("off2m", (128, DM), f32)    # off2 + 2**23
    wq = din("wq", (DM, H * DK), bf16)          # pre-scaled by 1/sqrt(DK)
    wk = din("wk", (DM, H * DK), bf16)
    wv = din("wv", (DM, H * DV), bf16)
    wo = din("wo", (H * DV, DM), bf16)
    w1 = din("w1", (DM, DI), bf16)
    w2 = din("w2", (DI, DM), bf16)
    g1 = din("g1", (128, DM), f32)
    g2 = din("g2", (128, DM), f32)

    enc_out = dout("enc_out", (NTPC, 128, DM), f32)
    sim_out = dout("sim_out", (NTPC, 128, S), f32)
    if dbg:
        dbg_en = dout("dbg_en", (128, DM), f32)
        dbg_mu = dout("dbg_mu", (128, DM), f32)
        dbg_encT = dout("dbg_encT", (128, 4, 128), bf16)
        dbg_kt = dout("dbg_kt", (128, 512), bf16)
        dbg_qt = dout("dbg_qt", (128, 512), bf16)
        dbg_vsb = dout("dbg_vsb", (128, 512), bf16)
        dbg_mst = dout("dbg_mst", (128, 256), bf16)
        dbg_aut = dout("dbg_aut", (128, 256), bf16)
        dbg_osb = dout("dbg_osb", (128, 512), bf16)
        dbg_y = dout("dbg_y", (128, DM), f32)
        dbg_enc1 = dout("dbg_enc1", (128, DM), f32)

    with tile.TileContext(nc) as tc:
      with tc.tile_pool(name="const", bufs=1) as cpool, \
           tc.tile_pool(name="persist", bufs=1) as ppool, \
           tc.tile_pool(name="dramp", bufs=1, space="DRAM") as dpool:

        def load_const(nm, ap, shape, dtype=f32):
            t = cpool.tile(list(shape), dtype, tag=nm, name=nm)
            nc.sync.dma_start(t[:], ap)
            return t

        tvg_s = load_const("c_tvg", tvals_g[:], (128, NGT))
        tvo_s = load_const("c_tvo", tvals_o[:], (128, NTPC))
        iv_s = load_const("c_iv", ivals[:], (128, NTPC))
        jk_s = load_const("c_jk", jkrow[:], (128, S))
        vm_s = load_const("c_vm", vmvals[:], (128, NTPC))
        ipv2_s = load_const("c_ipv2", ipv2[:], (128, DM))
        off2_s = load_const("c_off2", off2[:], (128, DM))
        off2m_s = load_const("c_off2m", off2m[:], (128, DM))
        g1_s = load_const("c_g1", g1[:], (128, DM))
        g2_s = load_const("c_g2", g2[:], (128, DM))
        ci_s = load_const("c_ci", col_idx[:], (128, 128), i16)
        ri_s = load_const("c_ri", row_idx[:], (128, NTPC), i32)
        evi_s = load_const("c_evi", ev_idx[:], (128, NGT), i32)
        vxi_s = load_const("c_vxi", vx_idx[:], (128, NGT), i32)
        evo_s = load_const("c_evo", ev_idx_o[:], (128, NTPC), i32)
        vxo_s = load_const("c_vxo", vx_idx_o[:], (128, NTPC), i32)

        ident_f = cpool.tile([128, 128], f32)
        make_identity(nc, ident_f[:])
        ident_b = cpool.tile([128, 128], bf16)
        make_identity(nc, ident_b[:])
        ones_b = cpool.tile([128, 1], bf16)
        nc.gpsimd.memset(ones_b[:], 1.0)
        eps_s = cpool.tile([128, 1], f32)
        nc.gpsimd.memset(eps_s[:], 1e-5)

        # persistent activations
        kt = [ppool.tile([128, S], bf16, tag=f"kt{h}", name=f"kt{h}")
              for h in range(H)]
        qt = [ppool.tile([128, NTPC * 128], bf16, tag=f"qt{h}", name=f"qt{h}")
              for h in range(H)]
        v_sb = ppool.tile([128, H, 4, 512], bf16)   # [j_loc, h, grp, (k,e)]
        res_d = dpool.tile([NTPC, 128, DM], f32)    # own enc rows (residual)

        # =================== Phase A: enc, Q/K/V ===================
        with tc.tile_pool(name="phA", bufs=1) as apool, \
             tc.tile_pool(name="phA2", bufs=2) as apool2, \
             tc.tile_pool(name="phAg", bufs=3) as gpool, \
             tc.tile_pool(name="psA", bufs=1, space="PSUM") as psA, \
             tc.tile_pool(name="psAt", bufs=2, space="PSUM") as psAt:

            wq_s = apool.tile([128, 4, H * DK], bf16)
            nc.sync.dma_start(wq_s[:], wq.rearrange("(c p) d -> p c d", p=128))
            wk_s = apool.tile([128, 4, H * DK], bf16)
            nc.sync.dma_start(wk_s[:], wk.rearrange("(c p) d -> p c d", p=128))
            wv_s = apool.tile([128, 4, H * DV], bf16)
            nc.sync.dma_start(wv_s[:], wv.rearrange("(c p) d -> p c d", p=128))

            encT = apool.tile([128, 4, S], bf16)          # enc^T (global)
            encOwnT = apool.tile([128, 4, NTPC * 128], bf16)

            def build_enc(n_tiles, evi, vxi, tv, encT_dst, own):
                # gathers in groups of 4 tiles (512 tokens)
                for g in range(n_tiles // 4):
                    ev_g = gpool.tile([128, 4, DM], f32, tag="embg")
                    vx_g = gpool.tile([128, 4, DM], f32, tag="embg")
                    for k4 in range(4):
                        # [128, 1] per-partition offsets: wider offset APs
                        # mis-gather on HW SWDGE
                        nc.gpsimd.indirect_dma_start(
                            ev_g[:, k4, :], None, ev_emb,
                            bass.IndirectOffsetOnAxis(
                                ap=evi[:, g * 4 + k4:g * 4 + k4 + 1], axis=0))
                        nc.gpsimd.indirect_dma_start(
                            vx_g[:, k4, :], None, vx_emb,
                            bass.IndirectOffsetOnAxis(
                                ap=vxi[:, g * 4 + k4:g * 4 + k4 + 1], axis=0))
                    for k in range(4):
                        lt = g * 4 + k
                        # temporal enc with range reduction in "turns":
                        # u = t*ipv2 + off2; sin(2*pi*u) with
                        # u - round(u) in [-0.5, 0.5] via the 2^23 trick.
                        MGC = 8388608.0
                        ru = apool2.tile([128, DM], f32, tag="ru")
                        nc.vector.scalar_tensor_tensor(
                            ru[:], ipv2_s[:], tv[:, lt:lt + 1], off2_s[:],
                            Alu.mult, Alu.add)
                        kv = apool2.tile([128, DM], f32, tag="kv")
                        nc.vector.tensor_scalar(
                            kv[:], ru[:], MGC, MGC, Alu.add, Alu.subtract)
                        mu = apool2.tile([128, DM], f32, tag="mu")
                        nc.vector.tensor_sub(mu[:], ru[:], kv[:])
                        tem = apool2.tile([128, DM], f32, tag="tem")
                        nc.scalar.activation(
                            tem[:], mu[:], Act.Sin,
                            scale=float(2 * np.pi * (1 - 1e-6)))
                        en = apool2.tile([128, DM], f32, tag="encn")
                        nc.vector.scalar_tensor_tensor(
                            en[:], ev_g[:, k, :], 0.0, vx_g[:, k, :],
                            Alu.add, Alu.add)
                        nc.vector.tensor_add(en[:], en[:], tem[:])
                        if own:
                            nc.sync.dma_start(res_d[lt], en[:])
                            if dbg and lt == 0:
                                nc.sync.dma_start(dbg_en, en[:])
                                nc.sync.dma_start(dbg_mu, mu[:])
                        # transpose -> encT chunk columns
                        tp = psAt.tile([128, 512], f32, tag="tpA")
                        for mc in range(4):
                            nc.tensor.transpose(
                                tp[:, mc * 128:(mc + 1) * 128],
                                en[:, mc * 128:(mc + 1) * 128], ident_f[:])
                        nc.scalar.copy(
                            encT_dst[:, :, lt * 128:(lt + 1) * 128], tp[:])

            build_enc(NGT, evi_s, vxi_s, tvg_s, encT, own=False)
            build_enc(NTPC, evo_s, vxo_s, tvo_s, encOwnT, own=True)

            for h in range(H):
                # K^T[h] : [dk, S]
                ktp = psA.tile([128, S], f32, tag="big")
                for mc in range(4):
                    for n2 in range(4):
                        nc.tensor.matmul(
                            ktp[:, n2 * 512:(n2 + 1) * 512],
                            wk_s[:, mc, h * 128:(h + 1) * 128],
                            encT[:, mc, n2 * 512:(n2 + 1) * 512],
                            start=(mc == 0), stop=(mc == 3))
                nc.scalar.copy(kt[h][:], ktp[:])
                # V^T[h] then transpose to V (natural, chunked)
                vtp = psA.tile([128, S], f32, tag="big")
                for mc in range(4):
                    for n2 in range(4):
                        nc.tensor.matmul(
                            vtp[:, n2 * 512:(n2 + 1) * 512],
                            wv_s[:, mc, h * 128:(h + 1) * 128],
                            encT[:, mc, n2 * 512:(n2 + 1) * 512],
                            start=(mc == 0), stop=(mc == 3))
                vt_h = apool2.tile([128, S], bf16, tag="vt")
                nc.scalar.copy(vt_h[:], vtp[:])
                for g4 in range(4):
                    tp = psAt.tile([128, 512], bf16, tag="tpAb")
                    for k in range(4):
                        c = g4 * 4 + k
                        nc.tensor.transpose(
                            tp[:, k * 128:(k + 1) * 128],
                            vt_h[:, c * 128:(c + 1) * 128], ident_b[:])
                    nc.scalar.copy(v_sb[:, h, g4, :], tp[:])
                # Q^T[h] over own tokens only
                qtp = psA.tile([128, NTPC * 128], f32, tag="big")
                for mc in range(4):
                    for n2 in range(2):
                        nc.tensor.matmul(
                            qtp[:, n2 * 512:(n2 + 1) * 512],
                            wq_s[:, mc, h * 128:(h + 1) * 128],
                            encOwnT[:, mc, n2 * 512:(n2 + 1) * 512],
                            start=(mc == 0), stop=(mc == 3))
                nc.scalar.copy(qt[h][:], qtp[:])

            if dbg:
                nc.sync.dma_start(dbg_encT, encT[:, :, 0:128])
                nc.sync.dma_start(dbg_kt, kt[0][:, 0:512])
                nc.sync.dma_start(dbg_qt, qt[0][:, 0:512])
                nc.sync.dma_start(dbg_vsb, v_sb[:, 0, 0, :])

        # =================== Phase B: per own tile ===================
        with tc.tile_pool(name="phB", bufs=1) as bpool, \
             tc.tile_pool(name="phB2", bufs=2) as bpool2, \
             tc.tile_pool(name="phB3", bufs=2) as bpool3, \
             tc.tile_pool(name="psB", bufs=2, space="PSUM") as psB, \
             tc.tile_pool(name="psBs", bufs=2, space="PSUM") as psBs:

            wo_s = bpool.tile([128, 4, DM], bf16)
            nc.sync.dma_start(wo_s[:], wo.rearrange("(c p) d -> p c d", p=128))
            w1_s = bpool.tile([128, 4, DI], bf16)
            nc.sync.dma_start(w1_s[:], w1.rearrange("(c p) d -> p c d", p=128))
            w2_s = bpool.tile([128, 8, DM], bf16)
            nc.sync.dma_start(w2_s[:], w2.rearrange("(c p) d -> p c d", p=128))

            for s in range(NTPC):
                E = 256 * (s + 1)          # key extent bound (both parities)
                NCH = E // 128             # 128-col chunks
                # ---- pairwise similarity: row gather + column gather ----
                ga = bpool2.tile([128, NVP], f32, tag="gaw")
                nc.gpsimd.indirect_dma_start(
                    ga[:], None, Ac,
                    bass.IndirectOffsetOnAxis(ap=ri_s[:, s:s + 1], axis=0))
                sa = bpool2.tile([128, S], f32, tag="saw")
                nc.gpsimd.ap_gather(sa[:], ga[:], ci_s[:], 128, NVP, 1, S)
                gw = bpool2.tile([128, NVP], f32, tag="gaw")
                nc.gpsimd.indirect_dma_start(
                    gw[:], None, Wc,
                    bass.IndirectOffsetOnAxis(ap=ri_s[:, s:s + 1], axis=0))
                sw = bpool2.tile([128, S], f32, tag="saw")
                nc.gpsimd.ap_gather(sw[:], gw[:], ci_s[:], 128, NVP, 1, S)
                pt = bpool.tile([128, S], f32, tag="P")
                nc.vector.tensor_mul(pt[:], sa[:], sw[:])
                nc.sync.dma_start(sim_out[s], pt[:])

                # ---- additive mask: -1e9 where (j > i) or pad-key ----
                gneg = bpool.tile([128, S], f32, tag="gneg")
                nc.vector.tensor_scalar(
                    gneg[:, :E], jk_s[:, :E], iv_s[:, s:s + 1], NEG,
                    Alu.is_gt, Alu.mult)
                ms = bpool.tile([128, S], bf16, tag="ms")
                nc.vector.scalar_tensor_tensor(
                    ms[:, :E], pt[:, :E], 1.0, gneg[:, :E], Alu.mult, Alu.add)

                # ---- transpose mask+sim to [j, i] layout ----
                mst = bpool.tile([128, S], bf16, tag="mst")
                for g4 in range((NCH + 3) // 4):
                    w = min(512, E - g4 * 512)
                    tp = psB.tile([128, 512], bf16, tag="tp")
                    for k in range(w // 128):
                        c = g4 * 4 + k
                        nc.tensor.transpose(
                            tp[:, k * 128:(k + 1) * 128],
                            ms[:, c * 128:(c + 1) * 128], ident_b[:])
                    nc.scalar.copy(mst[:, g4 * 512:g4 * 512 + w], tp[:, :w])

                if dbg and s == 0:
                    nc.sync.dma_start(dbg_mst, mst[:, 0:256])
                res_t = bpool3.tile([128, DM], f32, tag="res")
                nc.sync.dma_start(res_t[:], res_d[s])

                o_sb = bpool3.tile([128, H * DV], bf16, tag="osb")
                for h in range(H):
                    # scores^T (+mask preload) in groups of 4 chunks
                    aut = bpool3.tile([128, S], bf16, tag="aut")
                    for g4 in range((NCH + 3) // 4):
                        w = min(512, E - g4 * 512)
                        sc = psBs.tile([128, 512], f32, tag="sc")
                        for k in range(w // 128):
                            c = g4 * 4 + k
                            sl = sc[:, k * 128:(k + 1) * 128]
                            nc.tensor.matmul(
                                sl, ident_b[:],
                                mst[:, c * 128:(c + 1) * 128],
                                start=True, stop=False)
                            nc.tensor.matmul(
                                sl, kt[h][:, c * 128:(c + 1) * 128],
                                qt[h][:, s * 128:(s + 1) * 128],
                                start=False, stop=True)
                        nc.scalar.activation(
                            aut[:, g4 * 512:g4 * 512 + w], sc[:, :w], Act.Exp)
                    # denominator and attn @ V
                    dnp = psB.tile([128, 1], f32, tag="dn", bufs=1)
                    ovp = psB.tile([128, 128], f32, tag="ov")
                    for c in range(NCH):
                        a_c = aut[:, c * 128:(c + 1) * 128]
                        nc.tensor.matmul(dnp[:], a_c, ones_b[:],
                                         start=(c == 0), stop=(c == NCH - 1))
                        nc.tensor.matmul(
                            ovp[:], a_c,
                            v_sb[:, h, c // 4,
                                 (c % 4) * 128:(c % 4 + 1) * 128],
                            start=(c == 0), stop=(c == NCH - 1))
                    dent = bpool3.tile([128, 1], f32, tag="dent")
                    nc.vector.tensor_scalar_add(dent[:], dnp[:], 1e-30)
                    recip = bpool3.tile([128, 1], f32, tag="recip")
                    nc.vector.reciprocal(recip[:], dent[:])
                    nc.scalar.activation(
                        o_sb[:, h * 128:(h + 1) * 128], ovp[:], Act.Copy,
                        scale=recip[:])
                    if dbg and s == 0 and h == 0:
                        nc.sync.dma_start(dbg_aut, aut[:, 0:256])

                # transpose o -> [e, i], project with Wo
                tp = psB.tile([128, 512], bf16, tag="tp")
                for h in range(H):
                    nc.tensor.transpose(
                        tp[:, h * 128:(h + 1) * 128],
                        o_sb[:, h * 128:(h + 1) * 128], ident_b[:])
                ot_sb = bpool3.tile([128, H * DV], bf16, tag="otsb")
                nc.scalar.copy(ot_sb[:], tp[:])
                if dbg and s == 0:
                    nc.sync.dma_start(dbg_osb, o_sb[:])
                pj = psBs.tile([128, 512], f32, tag="sc")
                for h in range(H):
                    nc.tensor.matmul(
                        pj[:], ot_sb[:, h * 128:(h + 1) * 128], wo_s[:, h, :],
                        start=(h == 0), stop=(h == 3))

                # y = attn_out + residual  (bo==0 structurally)
                y = bpool3.tile([128, DM], f32, tag="y", bufs=1)
                nc.vector.scalar_tensor_tensor(
                    y[:], pj[:], 0.0, res_t[:], Alu.add, Alu.add)

                def layernorm(src, gain, out, out_tag, vm_ap=None):
                    st = bpool3.tile([128, 1], f32, tag="lnsum")
                    nc.vector.tensor_reduce(st[:], src[:], AX.X, Alu.add)
                    mn = bpool3.tile([128, 1], f32, tag="lnmn")
                    nc.vector.tensor_scalar_mul(mn[:], st[:], 1.0 / DM)
                    cc = bpool3.tile([128, DM], f32, tag="lncc")
                    nc.vector.tensor_scalar(cc[:], src[:], mn[:], None,
                                            Alu.subtract)
                    sq = bpool3.tile([128, DM], f32, tag="lnsq", bufs=1)
                    ssq = bpool3.tile([128, 1], f32, tag="lnssq")
                    nc.vector.scalar_tensor_tensor(
                        sq[:], cc[:], 0.0, cc[:], Alu.add, Alu.mult,
                        accum_out=ssq[:])
                    std = bpool3.tile([128, 1], f32, tag="lnstd")
                    nc.scalar.activation(std[:], ssq[:], Act.Sqrt,
                                         bias=eps_s[:], scale=1.0 / DM)
                    rstd = bpool3.tile([128, 1], f32, tag="lnrstd")
                    nc.vector.reciprocal(rstd[:], std[:])
                    if vm_ap is not None:
                        nc.vector.tensor_mul(rstd[:], rstd[:], vm_ap)
                    ot = bpool3.tile([128, DM], f32, tag=out_tag)
                    nc.vector.scalar_tensor_tensor(
                        ot[:], cc[:], rstd[:], gain[:], Alu.mult, Alu.mult)
                    return ot

                if dbg and s == 0:
                    nc.sync.dma_start(dbg_y, y[:])
                enc1 = layernorm(y, g1_s, None, "enc1")
                if dbg and s == 0:
                    nc.sync.dma_start(dbg_enc1, enc1[:])

                # FFN: transpose enc1, @w1, relu, transpose, @w2
                tp = psB.tile([128, 512], f32, tag="tpf", bufs=1)
                for mc in range(4):
                    nc.tensor.transpose(
                        tp[:, mc * 128:(mc + 1) * 128],
                        enc1[:, mc * 128:(mc + 1) * 128], ident_f[:])
                e1t = bpool3.tile([128, DM], bf16, tag="e1t")
                nc.scalar.copy(e1t[:], tp[:])
                f16t = bpool3.tile([128, DI], bf16, tag="f16")
                for half in range(2):
                    fp = psBs.tile([128, 512], f32, tag="sc")
                    for mc in range(4):
                        nc.tensor.matmul(
                            fp[:], e1t[:, mc * 128:(mc + 1) * 128],
                            w1_s[:, mc, half * 512:(half + 1) * 512],
                            start=(mc == 0), stop=(mc == 3))
                    nc.scalar.activation(
                        f16t[:, half * 512:(half + 1) * 512], fp[:], Act.Relu)
                ft = bpool3.tile([128, DI], bf16, tag="ft")
                for g4 in range(2):
                    tp = psB.tile([128, 512], bf16, tag="tp")
                    for k in range(4):
                        uc = g4 * 4 + k
                        nc.tensor.transpose(
                            tp[:, k * 128:(k + 1) * 128],
                            f16t[:, uc * 128:(uc + 1) * 128], ident_b[:])
                    nc.scalar.copy(ft[:, g4 * 512:(g4 + 1) * 512], tp[:])
                gp = psBs.tile([128, 512], f32, tag="sc")
                for uc in range(8):
                    nc.tensor.matmul(
                        gp[:], ft[:, uc * 128:(uc + 1) * 128], w2_s[:, uc, :],
                        start=(uc == 0), stop=(uc == 7))
                z = bpool3.tile([128, DM], f32, tag="z", bufs=1)
                nc.vector.scalar_tensor_tensor(
                    z[:], gp[:], 0.0, enc1[:], Alu.add, Alu.add)

                out_t = layernorm(z, g2_s, None, "outt",
                                  vm_ap=vm_s[:, s:s + 1])
                nc.sync.dma_start(enc_out[s], out_t[:])

    nc.compile()
    return nc


# ---------------------------------------------------------------------------
# host-side prep: build one core's input map
# ---------------------------------------------------------------------------

def _prep_shared(inputs):
    A = np.asarray(inputs["A"], np.float32)
    W = np.asarray(inputs["W"], np.float32)
    sh = {}
    sh["ev_emb"] = np.ascontiguousarray(np.asarray(inputs["event_emb"],
                                                   np.float32))
    sh["vx_emb"] = np.ascontiguousarray(np.asarray(inputs["vertex_emb"],
                                                   np.float32))
    rt = np.sqrt(np.float32(DK))
    sh["wq"] = (np.asarray(inputs["Wq"], np.float32) / rt).astype(BF16)
    sh["wk"] = np.asarray(inputs["Wk"], np.float32).astype(BF16)
    sh["wv"] = np.asarray(inputs["Wv"], np.float32).astype(BF16)
    sh["wo"] = np.asarray(inputs["Wo"], np.float32).astype(BF16)
    sh["w1"] = np.asarray(inputs["w1"], np.float32).astype(BF16)
    sh["w2"] = np.asarray(inputs["w2"], np.float32).astype(BF16)
    sh["g1"] = np.ascontiguousarray(
        np.broadcast_to(np.asarray(inputs["ln1_g"], np.float32), (128, DM)))
    sh["g2"] = np.ascontiguousarray(
        np.broadcast_to(np.asarray(inputs["ln2_g"], np.float32), (128, DM)))
    for nm in ("bo", "b1", "b2", "ln1_b", "ln2_b"):
        assert np.allclose(np.asarray(inputs[nm]), 0.0), \
            f"kernel assumes structurally-zero {nm}"
    i = np.arange(DM)
    pv = np.power(np.float32(10000.0),
                  (2.0 * (i // 2) / DM).astype(np.float32))
    ipv2 = (1.0 / (pv.astype(np.float64) * 2 * np.pi)).astype(np.float32)
    off2 = np.where(i % 2 == 0, np.float32(0.0),
                    np.float32(0.25)).astype(np.float32)
    sh["ipv2"] = np.ascontiguousarray(np.broadcast_to(ipv2, (128, DM)))
    sh["off2"] = np.ascontiguousarray(np.broadcast_to(off2, (128, DM)))
    sh["off2m"] = np.ascontiguousarray(
        np.broadcast_to(off2 + np.float32(8388608.0), (128, DM)))
    return A, W, sh


def _prep_core(inputs, A, W, shared, b, par):
    et = np.asarray(inputs["event_type"]).astype(np.int64)[b]
    vx = np.asarray(inputs["vertex"]).astype(np.int64)[b]
    t = np.asarray(inputs["event_time"], np.float32)[b]
    npm = np.asarray(inputs["non_pad_mask"], np.float32)[b, :, 0]

    idx = np.clip(vx - 1, 0, None)
    own_tiles = [2 * s + par for s in range(NTPC)]
    own_rows = np.concatenate([np.arange(gt * 128, (gt + 1) * 128)
                               for gt in own_tiles])

    # compacted A/W rows for this core's queries (zero row for pad vertices)
    rowids = np.where(vx[own_rows] != 0, idx[own_rows], -1)
    ur, inv = np.unique(rowids, return_inverse=True)
    R = len(ur)
    assert R <= RMAX
    Ac = np.zeros((RMAX, NVP), np.float32)
    Wc = np.zeros((RMAX, NVP), np.float32)
    real = ur >= 0
    Ac[np.nonzero(real)[0], :NV] = 10.0 * A[ur[real]]
    Wc[np.nonzero(real)[0], :NV] = W[ur[real]]

    d = dict(shared)
    d["Ac"], d["Wc"] = Ac, Wc
    d["row_idx"] = np.ascontiguousarray(
        inv.reshape(NTPC, 128).T.astype(np.int32))
    colids = np.where(vx != 0, idx, ZCOL)
    d["col_idx"] = _wrap16(colids)
    d["ev_idx"] = np.ascontiguousarray(et.reshape(NGT, 128).T.astype(np.int32))
    d["vx_idx"] = np.ascontiguousarray(vx.reshape(NGT, 128).T.astype(np.int32))
    d["ev_idx_o"] = np.ascontiguousarray(
        et[own_rows].reshape(NTPC, 128).T.astype(np.int32))
    d["vx_idx_o"] = np.ascontiguousarray(
        vx[own_rows].reshape(NTPC, 128).T.astype(np.int32))
    d["tvals_g"] = np.ascontiguousarray(t.reshape(NGT, 128).T)
    d["tvals_o"] = np.ascontiguousarray(t[own_rows].reshape(NTPC, 128).T)
    d["ivals"] = np.ascontiguousarray(
        own_rows.reshape(NTPC, 128).T.astype(np.float32))
    d["jkrow"] = np.ascontiguousarray(np.broadcast_to(
        np.where(et != 0, np.arange(S), 3.0e9).astype(np.float32), (128, S)))
    d["vmvals"] = np.ascontiguousarray(
        npm[own_rows].reshape(NTPC, 128).T.astype(np.float32))
    return d


# ---------------------------------------------------------------------------
# entry point
# ---------------------------------------------------------------------------

def _get_runner():
    """Build the SPMD program once and wrap it in a reusable 8-core
    jitted executor (mirrors concourse.bass2jax.run_bass_via_pjrt, but the
    jit is cached so repeat calls don't re-lower)."""
    if "runner" in _cache:
        return _cache["runner"]
    import jax
    from jax.sharding import Mesh, PartitionSpec
    from jax.experimental.shard_map import shard_map
    import concourse.mybir as mybir
    from concourse.bass2jax import (_bass_exec_p, install_neuronx_cc_hook,
                                    partition_id_tensor)

    nc = _build_program()
    install_neuronx_cc_hook()
    part_name = (nc.partition_id_tensor.name
                 if nc.partition_id_tensor else None)
    dbg_name = nc.dbg_addr.name if nc.dbg_addr is not None else None

    in_names, out_names, out_avals, zero_outs = [], [], [], []
    fill_zero = {}
    for alloc in nc.m.functions[0].allocations:
        if not isinstance(alloc, mybir.MemoryLocationSet):
            continue
        name = alloc.memorylocations[0].name
        if alloc.kind == "ExternalInput":
            if name == part_name:
                continue
            in_names.append(name)
            if name == dbg_name:
                # uint32[1,2] zero view of the unused 8-byte debug PA
                fill_zero[name] = np.zeros((1, 2), np.uint32)
        elif alloc.kind == "ExternalOutput":
            shape = tuple(alloc.tensor_shape)
            dtype = mybir.dt.np(alloc.dtype)
            out_names.append(name)
            out_avals.append(jax.core.ShapedArray(shape, dtype))
            zero_outs.append(np.zeros(shape, dtype))
    n_params = len(in_names)
    all_in_names = tuple(in_names + out_names
                         + ([part_name] if part_name else []))
    donate = tuple(range(n_params, n_params + len(out_names)))

    def _body(*args):
        operands = list(args)
        if part_name:
            operands.append(partition_id_tensor())
        return tuple(_bass_exec_p.bind(
            *operands,
            out_avals=tuple(out_avals),
            in_names=all_in_names,
            out_names=tuple(out_names),
            lowering_input_output_aliases=(),
            sim_require_finite=True,
            sim_require_nnan=True,
            nc=nc,
        ))

    devices = jax.devices()[:8]
    mesh = Mesh(np.asarray(devices), ("core",))
    in_specs = (PartitionSpec("core"),) * (n_params + len(out_names))
    out_specs = (PartitionSpec("core"),) * len(out_names)
    sharded = jax.jit(
        shard_map(_body, mesh=mesh, in_specs=in_specs, out_specs=out_specs,
                  check_rep=False),
        donate_argnums=donate, keep_unused=True)
    runner = dict(nc=nc, sharded=sharded, in_names=in_names,
                  out_names=out_names, out_avals=out_avals,
                  zero_outs=zero_outs, mesh=mesh, fill_zero=fill_zero)
    _cache["runner"] = runner
    return runner


def _concat_inputs(runner, in_maps):
    fz = runner["fill_zero"]
    return [np.concatenate(
        [np.asarray(m[name]) if name in m else fz[name] for m in in_maps],
        axis=0) for name in runner["in_names"]]


def _concat_zeros(runner):
    return [np.zeros((8 * z.shape[0], *z.shape[1:]), z.dtype)
            for z in runner["zero_outs"]]


def _run(runner, in_maps):
    out_arrs = runner["sharded"](*_concat_inputs(runner, in_maps),
                                 *_concat_zeros(runner))
    out = []
    for c in range(8):
        out.append({name: np.asarray(out_arrs[i]).reshape(
            8, *runner["out_avals"][i].shape)[c]
            for i, name in enumerate(runner["out_names"])})
    return out


def kernel(**inputs):
    runner = _get_runner()
    A, W, shared = _prep_shared(inputs)
    in_maps = [_prep_core(inputs, A, W, shared, c // 2, c % 2)
               for c in range(8)]
    results = _run(runner, in_maps)
    _cache["last_in_maps"] = in_maps

    enc = np.zeros((B, S, DM), np.float32)
    sim = np.zeros((B, 1, S, S), np.float32)
    for c in range(8):
        b, par = c // 2, c % 2
        eo = results[c]["enc_out"]
        so = results[c]["sim_out"]
        for s in range(NTPC):
            gt = 2 * s + par
            enc[b, gt * 128:(gt + 1) * 128] = eo[s]
            sim[b, 0, gt * 128:(gt + 1) * 128] = so[s]
    return enc, sim


def timeit_ns(iters=10):
    """Pipelined execution timing with device-resident inputs; returns
    per-iteration wall ns (upper bound on device exec time)."""
    import time
    import jax
    from jax.sharding import NamedSharding, PartitionSpec
    runner = _get_runner()
    in_maps = _cache["last_in_maps"]
    sh = NamedSharding(runner["mesh"], PartitionSpec("core"))
    dev_in = [jax.device_put(a, sh) for a in _concat_inputs(runner, in_maps)]
    zsets = [[jax.device_put(z, sh) for z in _concat_zeros(runner)]
             for _ in range(iters + 2)]
    for a in dev_in:
        a.block_until_ready()
    # warmup
    for w in range(2):
        outs = runner["sharded"](*dev_in, *zsets[w])
        for o in outs:
            o.block_until_ready()
    t0 = time.time()
    last = None
    for i in range(iters):
        last = runner["sharded"](*dev_in, *zsets[2 + i])
    for o in last:
        o.block_until_ready()
    dt = (time.time() - t0) / iters
    return dt * 1e9
